# revision 2
# baseline (speedup 1.0000x reference)
"""Trainium2 Bass kernel for a 2-layer LSTM (B=256, T=64, IN=96, H=1024, OUT=96).

Strategy: 8-way model-parallel over the gate/hidden dimension. Each core owns a
128-row slice of the hidden state per layer (and the corresponding 4x128 gate
rows), keeps its fp32 weight slices resident in SBUF, and streams the full batch
(N=256) through the PE array. After each cell update the 128x256 h-slice is
AllGathered so every core has the full h for the next step's contraction
(K=1024). A layer-skewed wavefront (layer 1 lags layer 0 by 2 steps, the output
projection by 4) gives every AllGather more than a full pipeline tick of slack,
so the collectives hide behind matmuls.

Layouts (per core j):
  h.T blocks      h0g/h1g[k]  [128, 256]   hidden-dim on partitions, batch free
  gate psum       [128, 512] bank tiles; chunk c in {i,f,g,o} -> own 128 rows
  weights         lhsT blocks [K=128, M=128] packed as [128, 4096] SBUF tensors
                  (column block (k*4+c)*128 + m), host pre-transposed/sliced
"""
import sys

sys.path.insert(0, "/opt/trn_rl_repo")
import numpy as np
import concourse.bacc as bacc
import concourse.mybir as mybir
import concourse.tile as tile
from concourse import bass_utils

NCORES = 8
B, T_FULL, IN, H, OUT = 256, 64, 96, 1024, 96
KB = H // 128  # 8 K-blocks per hidden contraction
F32 = mybir.dt.float32
SIG = mybir.ActivationFunctionType.Sigmoid
TANH = mybir.ActivationFunctionType.Tanh

_CACHE: dict = {}


def build(T=T_FULL, reps=1):
    """Build + compile the SPMD program. reps>1 repeats the whole recurrence
    (only rep 0 writes the external output) for slope-based timing."""
    key = (T, reps)
    if key in _CACHE:
        return _CACHE[key]

    nc = bacc.Bacc("TRN2", num_devices=NCORES, debug=False)

    d_w0 = nc.dram_tensor("w0", [128, 4 * KB * 128], F32, kind="ExternalInput")
    d_w1i = nc.dram_tensor("w1i", [128, 4 * KB * 128], F32, kind="ExternalInput")
    d_w1h = nc.dram_tensor("w1h", [128, 4 * KB * 128], F32, kind="ExternalInput")
    d_wih0 = nc.dram_tensor("wih0", [IN, 512], F32, kind="ExternalInput")
    d_wout = nc.dram_tensor("wout", [128, KB * OUT], F32, kind="ExternalInput")
    d_b0 = nc.dram_tensor("b0", [128, 4], F32, kind="ExternalInput")
    d_b1 = nc.dram_tensor("b1", [128, 4], F32, kind="ExternalInput")
    d_bout = nc.dram_tensor("bout", [OUT, 1], F32, kind="ExternalInput")
    d_xT = nc.dram_tensor("xT", [T, IN, B], F32, kind="ExternalInput")
    d_hinit = nc.dram_tensor("hinit", [2, H, B], F32, kind="ExternalInput")
    d_cinit = nc.dram_tensor("cinit", [2, 128, B], F32, kind="ExternalInput")
    d_out = nc.dram_tensor("outT", [T, OUT, B], F32, kind="ExternalOutput")

    with tile.TileContext(nc) as tc:
        with (
            tc.tile_pool(name="wp", bufs=1) as wp,
            tc.tile_pool(name="hg", bufs=1) as hgp,
            tc.tile_pool(name="cst", bufs=2) as cst,
            tc.tile_pool(name="cellp", bufs=2) as cellp,
            tc.tile_pool(name="xio", bufs=3) as xio,
            tc.tile_pool(name="hnp", bufs=2) as hnp,
            tc.tile_pool(name="psg0", bufs=1, space="PSUM") as psg0,
            tc.tile_pool(name="psg1", bufs=2, space="PSUM") as psg1,
            tc.tile_pool(name="pso", bufs=2, space="PSUM") as pso,
            tc.tile_pool(name="agd", bufs=2, space="DRAM") as agd,
        ):
            # ---- resident weights ----
            w0_s = wp.tile([128, 4 * KB * 128], F32, name="w0_s")
            w1i_s = wp.tile([128, 4 * KB * 128], F32, name="w1i_s")
            w1h_s = wp.tile([128, 4 * KB * 128], F32, name="w1h_s")
            for w_s, d_w in ((w0_s, d_w0), (w1i_s, d_w1i), (w1h_s, d_w1h)):
                for q in range(4):
                    sl = slice(q * 1024, (q + 1) * 1024)
                    nc.sync.dma_start(w_s[:, sl], d_w.ap()[:, sl])
            wih0_s = wp.tile([IN, 512], F32, name="wih0_s")
            nc.sync.dma_start(wih0_s[:], d_wih0.ap()[:])
            wout_s = wp.tile([128, KB * OUT], F32, name="wout_s")
            nc.sync.dma_start(wout_s[:], d_wout.ap()[:])
            b0_s = wp.tile([128, 4], F32, name="b0_s")
            nc.sync.dma_start(b0_s[:], d_b0.ap()[:])
            b1_s = wp.tile([128, 4], F32, name="b1_s")
            nc.sync.dma_start(b1_s[:], d_b1.ap()[:])
            bout_s = wp.tile([OUT, 1], F32, name="bout_s")
            nc.sync.dma_start(bout_s[:], d_bout.ap()[:])

            # ---- gathered-h tiles, 3-deep rotation (step mod 3) ----
            h0g = {}
            h1g = {}
            for par in range(3):
                h0g[par] = [
                    hgp.tile([128, B], F32, tag=f"h0g_{par}_{k}", name=f"h0g_{par}_{k}")
                    for k in range(KB)
                ]
                h1g[par] = [
                    hgp.tile([128, B], F32, tag=f"h1g_{par}_{k}", name=f"h1g_{par}_{k}")
                    for k in range(KB)
                ]
            for k in range(KB):
                ksl = slice(k * 128, (k + 1) * 128)
                nc.sync.dma_start(h0g[2][k][:], d_hinit.ap()[0, ksl, :])
                nc.sync.dma_start(h1g[2][k][:], d_hinit.ap()[1, ksl, :])

            c0_prev = cst.tile([128, B], F32, tag="c0", name="c0_init")
            nc.sync.dma_start(c0_prev[:], d_cinit.ap()[0])
            c1_prev = cst.tile([128, B], F32, tag="c1", name="c1_init")
            nc.sync.dma_start(c1_prev[:], d_cinit.ap()[1])

            def mm_block(ps_pair, c):
                tA, tB = ps_pair
                t = tA if c < 2 else tB
                off = (c % 2) * B
                return t[:, off : off + B]

            def cell(ps_pair, b_s, c_prev, lab):
                """LSTM cell elementwise on [128, B] gate slices."""
                tI = cellp.tile([128, B], F32, tag=f"I{lab[0]}", name=f"I{lab}")
                nc.scalar.activation(tI[:], mm_block(ps_pair, 0), SIG, bias=b_s[:, 0:1])
                tF = cellp.tile([128, B], F32, tag=f"F{lab[0]}", name=f"F{lab}")
                nc.scalar.activation(tF[:], mm_block(ps_pair, 1), SIG, bias=b_s[:, 1:2])
                tG = cellp.tile([128, B], F32, tag=f"G{lab[0]}", name=f"G{lab}")
                nc.scalar.activation(tG[:], mm_block(ps_pair, 2), TANH, bias=b_s[:, 2:3])
                tO = cellp.tile([128, B], F32, tag=f"O{lab[0]}", name=f"O{lab}")
                nc.scalar.activation(tO[:], mm_block(ps_pair, 3), SIG, bias=b_s[:, 3:4])
                tFC = cellp.tile([128, B], F32, tag=f"FC{lab[0]}", name=f"FC{lab}")
                nc.vector.tensor_mul(tFC[:], tF[:], c_prev[:])
                tIG = cellp.tile([128, B], F32, tag=f"IG{lab[0]}", name=f"IG{lab}")
                nc.vector.tensor_mul(tIG[:], tI[:], tG[:])
                cn = cst.tile([128, B], F32, tag=f"c{lab[0]}", name=f"c{lab}")
                nc.vector.tensor_add(cn[:], tFC[:], tIG[:])
                tT = cellp.tile([128, B], F32, tag=f"T{lab[0]}", name=f"T{lab}")
                nc.scalar.activation(tT[:], cn[:], TANH)
                hn = hnp.tile([128, B], F32, tag=f"hn{lab[0]}", name=f"hn{lab}")
                nc.vector.tensor_mul(hn[:], tO[:], tT[:])
                return hn, cn

            def allgather(hn, dst_tiles, lab):
                agi = agd.tile([128, B], F32, tag=f"agi{lab[0]}", name=f"agi{lab}")
                nc.sync.dma_start(agi[:], hn[:])
                ago = agd.tile(
                    [H, B], F32, tag=f"ago{lab[0]}", addr_space="Shared",
                    name=f"ago{lab}",
                )
                nc.gpsimd.collective_compute(
                    "AllGather",
                    mybir.AluOpType.bypass,
                    replica_groups=[list(range(NCORES))],
                    ins=[agi.opt()],
                    outs=[ago.opt()],
                )
                for k in range(KB):
                    nc.sync.dma_start(dst_tiles[k][:], ago[k * 128 : (k + 1) * 128, :])

            for r in range(reps):
                for tk in range(T + 4):
                    # ---- layer 1, step s1 = tk-2 ----
                    if 2 <= tk < T + 2:
                        s1 = tk - 2
                        lab1 = f"1_{r}_{tk}"
                        g1A = psg1.tile([128, 2 * B], F32, tag="g1A", name=f"g1A_{lab1}")
                        g1B = psg1.tile([128, 2 * B], F32, tag="g1B", name=f"g1B_{lab1}")
                        # each chunk's full chain must close (stop=True) before
                        # the next chunk's start=True in the same PSUM bank --
                        # groups may not interleave within a 2KB zero region
                        for c in range(4):
                            for k in range(KB):
                                wsl = slice((k * 4 + c) * 128, (k * 4 + c + 1) * 128)
                                nc.tensor.matmul(
                                    mm_block((g1A, g1B), c),
                                    w1i_s[:, wsl],
                                    h0g[s1 % 3][k][:],
                                    start=(k == 0),
                                    stop=False,
                                )
                            for k in range(KB):
                                wsl = slice((k * 4 + c) * 128, (k * 4 + c + 1) * 128)
                                nc.tensor.matmul(
                                    mm_block((g1A, g1B), c),
                                    w1h_s[:, wsl],
                                    h1g[(s1 - 1) % 3][k][:],
                                    start=False,
                                    stop=(k == KB - 1),
                                )
                        h1n, c1_prev = cell((g1A, g1B), b1_s, c1_prev, lab1)
                        allgather(h1n, h1g[s1 % 3], lab1)

                    # ---- output projection, step so = tk-4 ----
                    if tk >= 4:
                        so = tk - 4
                        labo = f"o_{r}_{tk}"
                        po = pso.tile([OUT, B], F32, tag="po", name=f"po_{labo}")
                        for k in range(KB):
                            nc.tensor.matmul(
                                po[:],
                                wout_s[:, k * OUT : (k + 1) * OUT],
                                h1g[so % 3][k][:],
                                start=(k == 0),
                                stop=(k == KB - 1),
                            )
                        ot = xio.tile([OUT, B], F32, tag="ot", name=f"ot_{labo}")
                        nc.scalar.activation(ot[:], po[:], SIG, bias=bout_s[:, 0:1])
                        if r == 0:
                            nc.sync.dma_start(d_out.ap()[so], ot[:])
                        else:
                            scr = agd.tile(
                                [OUT, B], F32, tag="scr", name=f"scr_{labo}"
                            )
                            nc.sync.dma_start(scr[:], ot[:])

                    # ---- layer 0, step s0 = tk ----
                    if tk < T:
                        s0 = tk
                        lab0 = f"0_{r}_{tk}"
                        xt = xio.tile([IN, B], F32, tag="xt", name=f"xt_{lab0}")
                        nc.sync.dma_start(xt[:], d_xT.ap()[s0])
                        g0A = psg0.tile([128, 2 * B], F32, tag="g0A", name=f"g0A_{lab0}")
                        g0B = psg0.tile([128, 2 * B], F32, tag="g0B", name=f"g0B_{lab0}")
                        for c in range(4):
                            nc.tensor.matmul(
                                mm_block((g0A, g0B), c),
                                wih0_s[:, c * 128 : (c + 1) * 128],
                                xt[:],
                                start=True,
                                stop=False,
                            )
                            for k in range(KB):
                                wsl = slice((k * 4 + c) * 128, (k * 4 + c + 1) * 128)
                                nc.tensor.matmul(
                                    mm_block((g0A, g0B), c),
                                    w0_s[:, wsl],
                                    h0g[(s0 - 1) % 3][k][:],
                                    start=False,
                                    stop=(k == KB - 1),
                                )
                        h0n, c0_prev = cell((g0A, g0B), b0_s, c0_prev, lab0)
                        allgather(h0n, h0g[s0 % 3], lab0)

    nc.compile()
    _CACHE[key] = nc
    return nc


def _prep_inputs(inputs, T=T_FULL):
    x = np.asarray(inputs["inputs"], np.float32)[:, :T]
    hid = np.asarray(inputs["hiddens"], np.float32)
    cel = np.asarray(inputs["cells"], np.float32)
    W_ih0 = np.asarray(inputs["W_ih0"], np.float32)
    W_hh0 = np.asarray(inputs["W_hh0"], np.float32)
    W_ih1 = np.asarray(inputs["W_ih1"], np.float32)
    W_hh1 = np.asarray(inputs["W_hh1"], np.float32)
    W_out = np.asarray(inputs["W_out"], np.float32)
    b0 = np.asarray(inputs["b_ih0"], np.float32) + np.asarray(
        inputs["b_hh0"], np.float32
    )
    b1 = np.asarray(inputs["b_ih1"], np.float32) + np.asarray(
        inputs["b_hh1"], np.float32
    )
    b_out = np.asarray(inputs["b_out"], np.float32)

    xT = np.ascontiguousarray(x.transpose(1, 2, 0))  # [T, IN, B]
    hinit = np.ascontiguousarray(hid.transpose(0, 2, 1))  # [2, H, B]
    cinT = cel.transpose(0, 2, 1)  # [2, H, B]
    wout = np.ascontiguousarray(
        W_out.reshape(OUT, KB, 128).transpose(2, 1, 0).reshape(128, KB * OUT)
    )
    bout = np.ascontiguousarray(b_out.reshape(OUT, 1))

    def blk(W, j):  # [4H, H] -> [128, 4*KB*128] lhsT blocks for core j
        Wj = W.reshape(4, NCORES, 128, H)[:, j]  # [c, m, k]
        A = Wj.reshape(4, 128, KB, 128).transpose(3, 2, 0, 1)  # [p, k, c, m]
        return np.ascontiguousarray(A.reshape(128, 4 * KB * 128))

    in_maps = []
    for j in range(NCORES):
        wih0j = W_ih0.reshape(4, NCORES, 128, IN)[:, j]  # [c, m, p]
        wih0j = np.ascontiguousarray(wih0j.transpose(2, 0, 1).reshape(IN, 512))
        in_maps.append(
            {
                "w0": blk(W_hh0, j),
                "w1i": blk(W_ih1, j),
                "w1h": blk(W_hh1, j),
                "wih0": wih0j,
                "wout": wout,
                "b0": np.ascontiguousarray(
                    b0.reshape(4, NCORES, 128)[:, j].T
                ),
                "b1": np.ascontiguousarray(
                    b1.reshape(4, NCORES, 128)[:, j].T
                ),
                "bout": bout,
                "xT": xT,
                "hinit": hinit,
                "cinit": np.ascontiguousarray(
                    cinT[:, j * 128 : (j + 1) * 128, :]
                ),
            }
        )
    return in_maps


def run(inputs, T=T_FULL, reps=1):
    nc = build(T, reps)
    in_maps = _prep_inputs(inputs, T)
    r = bass_utils.run_bass_kernel_spmd(nc, in_maps, core_ids=list(range(NCORES)))
    outT = r.results[0]["outT"]  # [T, OUT, B]
    return np.ascontiguousarray(outT.transpose(2, 0, 1))


def kernel(**inputs):
    return run(inputs, T=T_FULL, reps=1)


# revision 11
# speedup vs baseline: 437.6798x; 437.6798x over previous
"""Trainium2 Bass kernel for a 2-layer LSTM (B=256, T=64, IN=96, H=1024, OUT=96).

Strategy: 8-way model-parallel over the gate/hidden dimension. Each core owns a
128-row slice of the hidden state per layer (and the corresponding 4x128 gate
rows), keeps its fp32 weight slices resident in SBUF, and streams the full batch
(N=256) through the PE array. After each cell update the 128x256 h-slice is
AllGathered so every core has the full h for the next step's contraction
(K=1024). A layer-skewed wavefront (layer 1 lags layer 0 by 2 steps, the output
projection by 4) gives every AllGather more than a full pipeline tick of slack,
so the collectives hide behind matmuls.

Layouts (per core j):
  h.T blocks      h0g/h1g[k]  [128, 256]   hidden-dim on partitions, batch free
  gate psum       [128, 512] bank tiles; chunk c in {i,f,g,o} -> own 128 rows
  weights         lhsT blocks [K=128, M=128] packed as [128, 4096] SBUF tensors
                  (column block (k*4+c)*128 + m), host pre-transposed/sliced
"""
import sys

sys.path.insert(0, "/opt/trn_rl_repo")
import numpy as np
import concourse.bacc as bacc
import concourse.mybir as mybir
import concourse.tile as tile
from concourse import bass_utils

NCORES = 8
B, T_FULL, IN, H, OUT = 256, 64, 96, 1024, 96
KB = H // 128  # 8 K-blocks per hidden contraction
F32 = mybir.dt.float32
F32R = mybir.dt.float32r
SIG = mybir.ActivationFunctionType.Sigmoid
TANH = mybir.ActivationFunctionType.Tanh

_CACHE: dict = {}


def build(T=T_FULL, reps=1, mm_relaxed=True, single_core=False, probe_no_gather=False, merged_ag=False):
    """Build + compile the SPMD program. reps>1 repeats the whole recurrence
    (only rep 0 writes the external output) for slope-based timing."""
    key = (T, reps, mm_relaxed, single_core, probe_no_gather, merged_ag)
    if key in _CACHE:
        return _CACHE[key]

    nc = bacc.Bacc("TRN2", num_devices=1 if single_core else NCORES, debug=False)

    MDT = F32R if mm_relaxed else F32  # dtype of all matmul operands

    d_w0 = nc.dram_tensor("w0", [128, 4 * KB * 128], F32, kind="ExternalInput")
    d_w1i = nc.dram_tensor("w1i", [128, 4 * KB * 128], F32, kind="ExternalInput")
    d_w1h = nc.dram_tensor("w1h", [128, 4 * KB * 128], F32, kind="ExternalInput")
    d_wih0 = nc.dram_tensor("wih0", [IN, 512], F32, kind="ExternalInput")
    d_wout = nc.dram_tensor("wout", [128, KB * OUT], F32, kind="ExternalInput")
    d_b0 = nc.dram_tensor("b0", [128, 4], F32, kind="ExternalInput")
    d_b1 = nc.dram_tensor("b1", [128, 4], F32, kind="ExternalInput")
    d_bout = nc.dram_tensor("bout", [OUT, 1], F32, kind="ExternalInput")
    d_xT = nc.dram_tensor("xT", [T, IN, B], F32, kind="ExternalInput")
    d_hinit = nc.dram_tensor("hinit", [2, H, B], F32, kind="ExternalInput")
    d_cinit = nc.dram_tensor("cinit", [2, 128, B], F32, kind="ExternalInput")
    d_out = nc.dram_tensor("outT", [T, OUT, B], F32, kind="ExternalOutput")

    with tile.TileContext(nc) as tc:
        with (
            tc.tile_pool(name="wp", bufs=1) as wp,
            tc.tile_pool(name="hg", bufs=1) as hgp,
            tc.tile_pool(name="cst", bufs=2) as cst,
            tc.tile_pool(name="cellp", bufs=2) as cellp,
            tc.tile_pool(name="xio", bufs=3) as xio,
            tc.tile_pool(name="hnp", bufs=2) as hnp,
            tc.tile_pool(name="psg0", bufs=1, space="PSUM") as psg0,
            tc.tile_pool(name="psg1", bufs=2, space="PSUM") as psg1,
            tc.tile_pool(name="pso", bufs=2, space="PSUM") as pso,
            tc.tile_pool(name="agd", bufs=2, space="DRAM") as agd,
        ):
            # ---- resident weights (DVE-rounded to MDT for the PE) ----
            w0_s = wp.tile([128, 4 * KB * 128], MDT, name="w0_s")
            w1i_s = wp.tile([128, 4 * KB * 128], MDT, name="w1i_s")
            w1h_s = wp.tile([128, 4 * KB * 128], MDT, name="w1h_s")
            for w_s, d_w in ((w0_s, d_w0), (w1i_s, d_w1i), (w1h_s, d_w1h)):
                for q in range(4):
                    sl = slice(q * 1024, (q + 1) * 1024)
                    stg = wp.tile([128, 1024], F32, tag="wstg", bufs=2,
                                  name=f"stg_{d_w.name}_{q}")
                    nc.sync.dma_start(stg[:], d_w.ap()[:, sl])
                    nc.vector.tensor_copy(w_s[:, sl], stg[:])
            wih0_stg = wp.tile([IN, 512], F32, name="wih0_stg")
            nc.sync.dma_start(wih0_stg[:], d_wih0.ap()[:])
            wih0_s = wp.tile([IN, 512], MDT, name="wih0_s")
            nc.vector.tensor_copy(wih0_s[:], wih0_stg[:])
            wout_stg = wp.tile([128, KB * OUT], F32, name="wout_stg")
            nc.sync.dma_start(wout_stg[:], d_wout.ap()[:])
            wout_s = wp.tile([128, KB * OUT], MDT, name="wout_s")
            nc.vector.tensor_copy(wout_s[:], wout_stg[:])
            b0_s = wp.tile([128, 4], F32, name="b0_s")
            nc.sync.dma_start(b0_s[:], d_b0.ap()[:])
            b1_s = wp.tile([128, 4], F32, name="b1_s")
            nc.sync.dma_start(b1_s[:], d_b1.ap()[:])
            bout_s = wp.tile([OUT, 1], F32, name="bout_s")
            nc.sync.dma_start(bout_s[:], d_bout.ap()[:])

            # ---- gathered-h tiles, 3-deep rotation (step mod 3) ----
            h0g = {}
            h1g = {}
            for par in range(3):
                h0g[par] = [
                    hgp.tile([128, B], MDT, tag=f"h0g_{par}_{k}", name=f"h0g_{par}_{k}")
                    for k in range(KB)
                ]
                h1g[par] = [
                    hgp.tile([128, B], MDT, tag=f"h1g_{par}_{k}", name=f"h1g_{par}_{k}")
                    for k in range(KB)
                ]
            for k in range(KB):
                ksl = slice(k * 128, (k + 1) * 128)
                for l, hg in ((0, h0g), (1, h1g)):
                    hstg = wp.tile([128, B], F32, tag="hstg", bufs=2,
                                   name=f"hstg_{l}_{k}")
                    nc.sync.dma_start(hstg[:], d_hinit.ap()[l, ksl, :])
                    nc.vector.tensor_copy(hg[2][k][:], hstg[:])

            c0_prev = cst.tile([128, B], F32, tag="c0", name="c0_init")
            nc.sync.dma_start(c0_prev[:], d_cinit.ap()[0])
            c1_prev = cst.tile([128, B], F32, tag="c1", name="c1_init")
            nc.sync.dma_start(c1_prev[:], d_cinit.ap()[1])

            def mm_block(ps_pair, c):
                tA, tB = ps_pair
                t = tA if c < 2 else tB
                off = (c % 2) * B
                return t[:, off : off + B]

            c0_box = [c0_prev]
            c1_box = [c1_prev]

            def cell(ps_pair, b_s, _unused, lab):
                """LSTM cell elementwise on [128, B] gate slices."""
                c_prev = c0_box[0] if lab[0] == "0" else c1_box[0]
                tI = cellp.tile([128, B], F32, tag=f"I{lab[0]}", name=f"I{lab}")
                nc.scalar.activation(tI[:], mm_block(ps_pair, 0), SIG, bias=b_s[:, 0:1])
                tF = cellp.tile([128, B], F32, tag=f"F{lab[0]}", name=f"F{lab}")
                nc.scalar.activation(tF[:], mm_block(ps_pair, 1), SIG, bias=b_s[:, 1:2])
                tG = cellp.tile([128, B], F32, tag=f"G{lab[0]}", name=f"G{lab}")
                nc.scalar.activation(tG[:], mm_block(ps_pair, 2), TANH, bias=b_s[:, 2:3])
                tO = cellp.tile([128, B], F32, tag=f"O{lab[0]}", name=f"O{lab}")
                nc.scalar.activation(tO[:], mm_block(ps_pair, 3), SIG, bias=b_s[:, 3:4])
                tFC = cellp.tile([128, B], F32, tag=f"FC{lab[0]}", name=f"FC{lab}")
                nc.vector.tensor_mul(tFC[:], tF[:], c_prev[:])
                tIG = cellp.tile([128, B], F32, tag=f"IG{lab[0]}", name=f"IG{lab}")
                nc.vector.tensor_mul(tIG[:], tI[:], tG[:])
                cn = cst.tile([128, B], F32, tag=f"c{lab[0]}", name=f"c{lab}")
                nc.vector.tensor_add(cn[:], tFC[:], tIG[:])
                tT = cellp.tile([128, B], F32, tag=f"T{lab[0]}", name=f"T{lab}")
                nc.scalar.activation(tT[:], cn[:], TANH)
                hn = hnp.tile([128, B], MDT, tag=f"hn{lab[0]}", name=f"hn{lab}")
                nc.vector.tensor_mul(hn[:], tO[:], tT[:])
                return hn, cn

            def allgather(hn, dst_tiles, lab):
                if probe_no_gather:
                    # timing probe: no DMA/collective, fill dst via DVE copies
                    for k in range(KB):
                        nc.vector.tensor_copy(dst_tiles[k][:], hn[:])
                    return
                agi = agd.tile([128, B], MDT, tag=f"agi{lab[0]}", name=f"agi{lab}")
                nc.sync.dma_start(agi[:], hn[:])
                ago = agd.tile(
                    [H, B], MDT, tag=f"ago{lab[0]}",
                    addr_space="Local" if single_core else "Shared",
                    name=f"ago{lab}",
                )
                if single_core:
                    pass  # timing stand-in: consumers read agi directly below
                else:
                    nc.gpsimd.collective_compute(
                        "AllGather",
                        mybir.AluOpType.bypass,
                        replica_groups=[list(range(NCORES))],
                        ins=[agi.opt()],
                        outs=[ago.opt()],
                    )
                for k in range(KB):
                    src_ap = agi[:] if single_core else ago[k * 128 : (k + 1) * 128, :]
                    nc.sync.dma_start(dst_tiles[k][:], src_ap)

            def do_collective(agi, ago, lab):
                if single_core:
                    return
                nc.gpsimd.collective_compute(
                    "AllGather",
                    mybir.AluOpType.bypass,
                    replica_groups=[list(range(NCORES))],
                    ins=[agi.opt()],
                    outs=[ago.opt()],
                )

            def emit_g1(tk, r, s1):
                lab1 = f"1_{r}_{tk}"
                g1A = psg1.tile([128, 2 * B], F32, tag="g1A", name=f"g1A_{lab1}")
                g1B = psg1.tile([128, 2 * B], F32, tag="g1B", name=f"g1B_{lab1}")
                for c in range(4):
                    for k in range(KB):
                        wsl = slice((k * 4 + c) * 128, (k * 4 + c + 1) * 128)
                        nc.tensor.matmul(
                            mm_block((g1A, g1B), c), w1i_s[:, wsl],
                            h0g[s1 % 3][k][:], start=(k == 0), stop=False,
                        )
                    for k in range(KB):
                        wsl = slice((k * 4 + c) * 128, (k * 4 + c + 1) * 128)
                        nc.tensor.matmul(
                            mm_block((g1A, g1B), c), w1h_s[:, wsl],
                            h1g[(s1 - 1) % 3][k][:], start=False, stop=(k == KB - 1),
                        )
                return cell((g1A, g1B), b1_s, None, lab1)

            def emit_g0(tk, r, s0):
                lab0 = f"0_{r}_{tk}"
                xt = xio.tile([IN, B], F32, tag="xt", name=f"xt_{lab0}")
                nc.sync.dma_start(xt[:], d_xT.ap()[s0])
                xt_r = xio.tile([IN, B], MDT, tag="xtr", name=f"xtr_{lab0}")
                nc.vector.tensor_copy(xt_r[:], xt[:])
                g0A = psg0.tile([128, 2 * B], F32, tag="g0A", name=f"g0A_{lab0}")
                g0B = psg0.tile([128, 2 * B], F32, tag="g0B", name=f"g0B_{lab0}")
                for c in range(4):
                    nc.tensor.matmul(
                        mm_block((g0A, g0B), c),
                        wih0_s[:, c * 128 : (c + 1) * 128], xt_r[:],
                        start=True, stop=False,
                    )
                    for k in range(KB):
                        wsl = slice((k * 4 + c) * 128, (k * 4 + c + 1) * 128)
                        nc.tensor.matmul(
                            mm_block((g0A, g0B), c), w0_s[:, wsl],
                            h0g[(s0 - 1) % 3][k][:], start=False, stop=(k == KB - 1),
                        )
                return cell((g0A, g0B), b0_s, None, lab0)

            def emit_po(tk, r, so, out_external):
                labo = f"o_{r}_{tk}"
                po = pso.tile([OUT, B], F32, tag="po", name=f"po_{labo}")
                for k in range(KB):
                    nc.tensor.matmul(
                        po[:], wout_s[:, k * OUT : (k + 1) * OUT],
                        h1g[so % 3][k][:], start=(k == 0), stop=(k == KB - 1),
                    )
                ot = xio.tile([OUT, B], F32, tag="ot", name=f"ot_{labo}")
                nc.scalar.activation(ot[:], po[:], SIG, bias=bout_s[:, 0:1])
                if out_external:
                    nc.sync.dma_start(d_out.ap()[so], ot[:])
                else:
                    scr = agd.tile([OUT, B], F32, tag="scr", name=f"scr_{labo}")
                    nc.sync.dma_start(scr[:], ot[:])

            for r in range(reps):
                if not merged_ag:
                    for tk in range(T + 4):
                        if 2 <= tk < T + 2:
                            s1 = tk - 2
                            h1n, c1_box[0] = emit_g1(tk, r, s1)
                            allgather(h1n, h1g[s1 % 3], f"1_{r}_{tk}")
                        if tk >= 4:
                            emit_po(tk, r, tk - 4, r == 0)
                        if tk < T:
                            s0 = tk
                            h0n, c0_box[0] = emit_g0(tk, r, s0)
                            allgather(h0n, h0g[s0 % 3], f"0_{r}_{tk}")
                else:
                    S1L, POL = 3, 5
                    for tk in range(T + POL):
                        h1n = h0n = None
                        if S1L <= tk < T + S1L:
                            h1n, c1_box[0] = emit_g1(tk, r, tk - S1L)
                        if POL <= tk < T + POL:
                            emit_po(tk, r, tk - POL, r == 0)
                        if tk < T:
                            h0n, c0_box[0] = emit_g0(tk, r, tk)
                        # ---- one merged AllGather per tick ----
                        lab = f"m_{r}_{tk}"
                        if probe_no_gather:
                            if h0n is not None:
                                for k in range(KB):
                                    nc.vector.tensor_copy(h0g[tk % 3][k][:], h0n[:])
                            if h1n is not None:
                                for k in range(KB):
                                    nc.vector.tensor_copy(
                                        h1g[(tk - S1L) % 3][k][:], h1n[:])
                            continue
                        nslc = (h0n is not None) + (h1n is not None)
                        if nslc == 0:
                            continue
                        agi = agd.tile([nslc * 128, B], MDT, tag=f"agi{nslc}",
                                       name=f"agi_{lab}")
                        off = 0
                        for hn in (h0n, h1n):
                            if hn is not None:
                                nc.sync.dma_start(agi[off : off + 128, :], hn[:])
                                off += 128
                        ago = agd.tile(
                            [nslc * H, B], MDT, tag=f"ago{nslc}",
                            addr_space="Local" if single_core else "Shared",
                            name=f"ago_{lab}",
                        )
                        do_collective(agi, ago, lab)
                        stride = nslc * 128
                        for k in range(KB):
                            base = k * stride
                            off = 0
                            if h0n is not None:
                                sap = (agi[0:128, :] if single_core
                                       else ago[base : base + 128, :])
                                nc.sync.dma_start(h0g[tk % 3][k][:], sap)
                                off = 128
                            if h1n is not None:
                                sap = (agi[off : off + 128, :] if single_core
                                       else ago[base + off : base + off + 128, :])
                                nc.sync.dma_start(h1g[(tk - S1L) % 3][k][:], sap)

    nc.compile()
    _CACHE[key] = nc
    return nc


def _prep_inputs(inputs, T=T_FULL):
    x = np.asarray(inputs["inputs"], np.float32)[:, :T]
    hid = np.asarray(inputs["hiddens"], np.float32)
    cel = np.asarray(inputs["cells"], np.float32)
    W_ih0 = np.asarray(inputs["W_ih0"], np.float32)
    W_hh0 = np.asarray(inputs["W_hh0"], np.float32)
    W_ih1 = np.asarray(inputs["W_ih1"], np.float32)
    W_hh1 = np.asarray(inputs["W_hh1"], np.float32)
    W_out = np.asarray(inputs["W_out"], np.float32)
    b0 = np.asarray(inputs["b_ih0"], np.float32) + np.asarray(
        inputs["b_hh0"], np.float32
    )
    b1 = np.asarray(inputs["b_ih1"], np.float32) + np.asarray(
        inputs["b_hh1"], np.float32
    )
    b_out = np.asarray(inputs["b_out"], np.float32)

    xT = np.ascontiguousarray(x.transpose(1, 2, 0))  # [T, IN, B]
    hinit = np.ascontiguousarray(hid.transpose(0, 2, 1))  # [2, H, B]
    cinT = cel.transpose(0, 2, 1)  # [2, H, B]
    wout = np.ascontiguousarray(
        W_out.reshape(OUT, KB, 128).transpose(2, 1, 0).reshape(128, KB * OUT)
    )
    bout = np.ascontiguousarray(b_out.reshape(OUT, 1))

    def blk(W, j):  # [4H, H] -> [128, 4*KB*128] lhsT blocks for core j
        Wj = W.reshape(4, NCORES, 128, H)[:, j]  # [c, m, k]
        A = Wj.reshape(4, 128, KB, 128).transpose(3, 2, 0, 1)  # [p, k, c, m]
        return np.ascontiguousarray(A.reshape(128, 4 * KB * 128))

    in_maps = []
    for j in range(NCORES):
        wih0j = W_ih0.reshape(4, NCORES, 128, IN)[:, j]  # [c, m, p]
        wih0j = np.ascontiguousarray(wih0j.transpose(2, 0, 1).reshape(IN, 512))
        in_maps.append(
            {
                "w0": blk(W_hh0, j),
                "w1i": blk(W_ih1, j),
                "w1h": blk(W_hh1, j),
                "wih0": wih0j,
                "wout": wout,
                "b0": np.ascontiguousarray(
                    b0.reshape(4, NCORES, 128)[:, j].T
                ),
                "b1": np.ascontiguousarray(
                    b1.reshape(4, NCORES, 128)[:, j].T
                ),
                "bout": bout,
                "xT": xT,
                "hinit": hinit,
                "cinit": np.ascontiguousarray(
                    cinT[:, j * 128 : (j + 1) * 128, :]
                ),
            }
        )
    return in_maps


def run(inputs, T=T_FULL, reps=1, mm_relaxed=True):
    nc = build(T, reps, mm_relaxed)
    in_maps = _prep_inputs(inputs, T)
    r = bass_utils.run_bass_kernel_spmd(nc, in_maps, core_ids=list(range(NCORES)))
    outT = r.results[0]["outT"]  # [T, OUT, B]
    return np.ascontiguousarray(outT.transpose(2, 0, 1))


def kernel(**inputs):
    return run(inputs, T=T_FULL, reps=1)
